# revision 2
# baseline (speedup 1.0000x reference)
"""Trainium2 Bass kernel for the 4-group sparse-tap 3x3 conv.

Computation (see reference): x (32,128,56,56) f32, weights (32,2048) f32.
Four groups of 32 output channels; group g uses 4 taps CFG[g] of the 3x3
footprint over all 128 input channels. Output (32,128,56,56) f32.

Strategy: pure data-parallel over batch — 4 images per NeuronCore, 8 cores.

Host prep: zero-pad each image to 59x58 (1-pixel conv halo + one extra row
so the last shifted matmul view stays in-bounds), cast to fp16, and lay the
4 images of a core out channel-major ([128 ic, 4*59*58]) so shards DMA with
large contiguous per-partition descriptors.  Weights are rearranged into 16
[ic=128, oc=32] fp16 stationary blocks, one per (group, tap) pair.

Device, per image: for each chunk of 8 output rows, issue 16 column-tiled
matmuls (tile_position=(0,32g)) — group g's 4 taps accumulate into PSUM
partitions 32g..32g+31.  Tap (kr,kc) uses the rhs slice starting at
(8c+kr)*58+kc, which yields all 8 shifted rows in one contiguous view
thanks to the width padding.  The 4 groups' matmuls execute concurrently on
the PE's 32-column sub-arrays, so a chunk costs ~4 matmul streams instead
of 9 (the dense-tap formulation): ~800ns/chunk warm.  fp16 keeps 10
mantissa bits and accumulates in fp32; outputs are stored fp16 (DVE casts
on the PSUM->SBUF copy) and upcast to f32 on the host, halving output HBM
traffic (total DMA 10.0MB -> 6.8MB per core, under the ~358GB/s cap).

DMA plan: three rings (sync Q1, scalar Q10 HWDGE; gpsimd Q0 SWDGE), each
~145GB/s sustained.  Inputs are split into row-piece DMAs sized so compute
can start as soon as the first piece + weights land (~9.5us) instead of
waiting for a whole image: img0 in 4 pieces on sync, w+img2 on scalar,
img1+img3 on gpsimd.  Outputs stream out per-image as chunks are copied,
in compute order: out0 sync, out1 scalar, out2 gpsimd, and img3's chunks
fan out across all three rings so the final flush after the last matmul is
only ~1us.  A few dummy matmuls at the start lift the HAM clock gate
(1.2 -> 2.4 GHz) while the first pieces are in flight, and trailing dummy
matmuls (interleaved with waits on the output-DMA semaphores) keep the
clock boosted through the framework's postamble semaphore sweep, which
otherwise runs at half clock (~8us -> ~4us).
"""

from contextlib import ExitStack

import numpy as np

import concourse.bass as bass
import concourse.mybir as mybir
from concourse.bass_utils import run_bass_kernel_spmd

CFG = [[1, 2, 4, 5], [2, 3, 5, 6], [4, 5, 7, 8], [5, 6, 8, 9]]

B, C, H, W = 32, 128, 56, 56
NCORES = 8
BPC = B // NCORES            # images per core
HP, WP = H + 3, W + 2        # padded rows (1 top, 1 bottom, 1 overread), cols
XF = HP * WP                 # 3422 padded free elems per image
OF = H * W                   # 3136 output free elems per image
NPSUM = 8                    # psum banks cycled over chunks
RPC = 8                      # output rows per chunk
NCHUNK = H // RPC            # 7
NFREE = RPC * WP             # 464 matmul free dim
F32 = mybir.dt.float32
F16 = mybir.dt.float16
SLOT = [0, 1, 2, 0]          # output slot per image (3 slots)

# row-piece boundaries (in padded-row units) for the input splits
R0 = [0, 10, 26, 42, HP]     # img0: 4 pieces on sync
R2 = [0, 26, HP]             # imgs 1/2/3: 2 pieces each
CW = RPC * W                 # output cols per chunk (448)


def _build_nc():
    nc = bass.Bass()
    xp = nc.declare_dram_parameter("x", [C, BPC * XF], F16, isOutput=False)
    wp = nc.declare_dram_parameter("w", [C, 16 * 32], F16, isOutput=False)
    op = nc.declare_dram_parameter("out", [BPC, C, OF], F16, isOutput=True)

    with ExitStack() as ctx:
        w_tile = ctx.enter_context(nc.sbuf_tensor("w_tile", [C, 16 * 32], F16))
        xbuf = ctx.enter_context(nc.sbuf_tensor("xbuf", [C, BPC * XF], F16))
        o_slots = [ctx.enter_context(nc.sbuf_tensor(f"o_slot{i}", [C, OF], F16))
                   for i in range(3)]
        psums = [ctx.enter_context(nc.psum_tensor(f"psum{i}", [C, 512], F32))
                 for i in range(NPSUM)]

        x_sync = ctx.enter_context(nc.semaphore("x_sync"))
        x_sc = ctx.enter_context(nc.semaphore("x_sc"))
        x_gp = ctx.enter_context(nc.semaphore("x_gp"))
        o_sync = ctx.enter_context(nc.semaphore("o_sync"))
        o_sc = ctx.enter_context(nc.semaphore("o_sc"))
        o_gp = ctx.enter_context(nc.semaphore("o_gp"))
        mm_sem = ctx.enter_context(nc.semaphore("mm_sem"))
        v_sem = ctx.enter_context(nc.semaphore("v_sem"))

        block = ctx.enter_context(nc.Block(no_gpsimd_drain=True))

        def img_piece(dma, b, r_lo, r_hi, sem):
            lo, hi = b * XF + r_lo * WP, b * XF + r_hi * WP
            dma.dma_start(out=xbuf[:, lo:hi], in_=xp[:, lo:hi]).then_inc(sem, 16)

        def out_piece(dma, b, c_lo, c_hi, sem):
            dma.dma_start(
                out=op[b][:, c_lo * CW:c_hi * CW],
                in_=o_slots[SLOT[b]][:, c_lo * CW:c_hi * CW],
            ).then_inc(sem, 16)

        @block.sync
        def _(sync):
            for i in range(4):                      # img0 in 4 row pieces
                img_piece(sync, 0, R0[i], R0[i + 1], x_sync)
            sync.wait_ge(v_sem, 4)
            out_piece(sync, 0, 0, 4, o_sync)        # out0 c0-3
            sync.wait_ge(v_sem, 7)
            out_piece(sync, 0, 4, 7, o_sync)        # out0 c4-6
            sync.wait_ge(v_sem, 3 * NCHUNK + 6)
            out_piece(sync, 3, 4, 6, o_sync)        # out3 c4-5
            sync.wait_ge(o_sync, 48)

        @block.scalar
        def _(scalar):
            scalar.dma_start(out=w_tile[:], in_=wp[:]).then_inc(x_sc, 16)
            img_piece(scalar, 2, R2[0], R2[1], x_sc)
            img_piece(scalar, 2, R2[1], R2[2], x_sc)
            scalar.wait_ge(v_sem, NCHUNK + 4)
            out_piece(scalar, 1, 0, 4, o_sc)        # out1 c0-3
            scalar.wait_ge(v_sem, 2 * NCHUNK)
            out_piece(scalar, 1, 4, 7, o_sc)        # out1 c4-6
            scalar.wait_ge(v_sem, 3 * NCHUNK + 4)
            out_piece(scalar, 3, 2, 4, o_sc)        # out3 c2-3
            scalar.wait_ge(v_sem, 4 * NCHUNK)
            out_piece(scalar, 3, 6, 7, o_sc)        # out3 c6
            scalar.wait_ge(o_sc, 64)

        @block.gpsimd
        def _(gpsimd):
            img_piece(gpsimd, 1, R2[0], R2[1], x_gp)
            img_piece(gpsimd, 1, R2[1], R2[2], x_gp)
            img_piece(gpsimd, 3, R2[0], R2[1], x_gp)
            img_piece(gpsimd, 3, R2[1], R2[2], x_gp)
            gpsimd.wait_ge(v_sem, 2 * NCHUNK + 4)
            out_piece(gpsimd, 2, 0, 4, o_gp)        # out2 c0-3
            gpsimd.wait_ge(v_sem, 3 * NCHUNK)
            out_piece(gpsimd, 2, 4, 7, o_gp)        # out2 c4-6
            gpsimd.wait_ge(v_sem, 3 * NCHUNK + 2)
            out_piece(gpsimd, 3, 0, 2, o_gp)        # out3 c0-1
            gpsimd.wait_ge(o_gp, 48)

        @block.tensor
        def _(tensor):
            # dummy matmuls on garbage data: lift the HAM clock gate while
            # the first input pieces are still in flight.
            for _ in range(4):
                tensor.matmul(
                    psums[NPSUM - 1][0:32, :NFREE],
                    w_tile[:, 0:32],
                    xbuf[:, 0:NFREE],
                    start=True, stop=True,
                    tile_position=(0, 0),
                )
            tensor.wait_ge(x_sc, 16)        # weights
            tensor.wait_ge(x_sync, 16)      # img0 rows 0-9
            # (global chunk -> input-piece semaphore threshold) gates
            gates = {1: (x_sync, 32), 3: (x_sync, 48), 5: (x_sync, 64),
                     7: (x_gp, 16), 10: (x_gp, 32),
                     14: (x_sc, 32), 17: (x_sc, 48),
                     21: (x_gp, 48), 24: (x_gp, 64)}
            for b in range(BPC):
                for c in range(NCHUNK):
                    g = NCHUNK * b + c
                    if g in gates:
                        tensor.wait_ge(*gates[g])
                    if g >= NPSUM:
                        # psum bank g%NPSUM free once chunk g-NPSUM was copied
                        tensor.wait_ge(v_sem, g - NPSUM + 1)
                    bank = psums[g % NPSUM]
                    for j in range(4):
                        for grp in range(4):
                            t = CFG[grp][j]
                            kr, kc = (t - 1) // 3, (t - 1) % 3
                            off = b * XF + (RPC * c + kr) * WP + kc
                            idx = grp * 4 + j
                            mm = tensor.matmul(
                                bank[32 * grp:32 * (grp + 1), :NFREE],
                                w_tile[:, idx * 32:(idx + 1) * 32],
                                xbuf[:, off:off + NFREE],
                                start=(j == 0),
                                stop=(j == 3),
                                tile_position=(0, 32 * grp),
                            )
                    mm.then_inc(mm_sem, 1)
            # HAM keepalive: PE activity through the output flush keeps the
            # clock boosted for the framework postamble (bank 4 is dead by
            # v_sem>=21; these results are never read).
            for sem, val in ((v_sem, 4 * NCHUNK), (o_gp, 48),
                             (o_sync, 48), (o_sc, 64)):
                tensor.wait_ge(sem, val)
                tensor.matmul(
                    psums[4][0:32, :NFREE],
                    w_tile[:, 0:32],
                    xbuf[:, 0:NFREE],
                    start=True, stop=True,
                    tile_position=(0, 0),
                )

        @block.vector
        def _(vector):
            for b in range(BPC):
                if b == 3:
                    vector.wait_ge(o_sync, 32)   # out0 done -> slot0 free
                for c in range(NCHUNK):
                    g = NCHUNK * b + c
                    vector.wait_ge(mm_sem, g + 1)
                    src = psums[g % NPSUM][:, :NFREE].rearrange(
                        "p (r w) -> p r w", w=WP)[:, :, :W]
                    dst = o_slots[SLOT[b]][:, c * CW:(c + 1) * CW].rearrange(
                        "p (r w) -> p r w", w=W)
                    vector.tensor_copy(out=dst, in_=src).then_inc(v_sem, 1)

    return nc


_NC_CACHE = None


def _get_nc():
    global _NC_CACHE
    if _NC_CACHE is None:
        _NC_CACHE = _build_nc()
    return _NC_CACHE


def _prep_weights(weights):
    """(32, 2048) grouped-sparse -> 16 [ic=128, oc=32] fp16 lhsT blocks."""
    w16 = np.zeros((C, 16 * 32), np.float32)
    for g, taps in enumerate(CFG):
        blk = np.asarray(weights[:, g * 512:(g + 1) * 512], np.float32)
        blk = blk.reshape(32, C, 4)  # [oc_in_group, ic, tap_j]
        for j in range(4):
            idx = g * 4 + j
            w16[:, idx * 32:(idx + 1) * 32] = blk[:, :, j].T
    return np.ascontiguousarray(w16.astype(np.float16))


def _prep_x(x):
    """(32,128,56,56) f32 -> per-core channel-major padded fp16 shards."""
    xpad = np.zeros((B, C, HP, WP), np.float16)
    xpad[:, :, 1:H + 1, 1:W + 1] = x.astype(np.float16)
    xs = xpad.reshape(NCORES, BPC, C, XF)
    # (core, b, c, f) -> (core, c, b*f)
    xs = np.ascontiguousarray(xs.transpose(0, 2, 1, 3)).reshape(NCORES, C, BPC * XF)
    return xs


def kernel(x, weights):
    x = np.asarray(x, np.float32)
    weights = np.asarray(weights, np.float32)

    xs = _prep_x(x)
    wflat = _prep_weights(weights)

    nc = _get_nc()
    in_maps = [{"x": xs[i], "w": wflat} for i in range(NCORES)]
    res = run_bass_kernel_spmd(nc, in_maps, core_ids=list(range(NCORES)))
    return np.concatenate(
        [res.results[i]["out"].astype(np.float32).reshape(BPC, C, H, W)
         for i in range(NCORES)],
        axis=0,
    )


# revision 9
# speedup vs baseline: 1.1104x; 1.1104x over previous
"""Trainium2 Bass kernel for the 4-group sparse-tap 3x3 conv.

Computation (see reference): x (32,128,56,56) f32, weights (32,2048) f32.
Four groups of 32 output channels; group g uses 4 taps CFG[g] of the 3x3
footprint over all 128 input channels. Output (32,128,56,56) f32.

Strategy: pure data-parallel over batch — 4 images per NeuronCore, 8 cores.

Host prep: zero-pad each image to 59x58 (1-pixel conv halo + one extra row
so the last shifted matmul view stays in-bounds), cast to fp16, and lay the
4 images of a core out channel-major ([128 ic, 4*59*58]) so shards DMA with
large contiguous per-partition descriptors.  Weights are rearranged into 16
[ic=128, oc=32] fp16 stationary blocks, one per (group, tap) pair.

Device, per image: for each chunk of 8 output rows, issue 16 column-tiled
matmuls (tile_position=(0,32g)) — group g's 4 taps accumulate into PSUM
partitions 32g..32g+31.  Tap (kr,kc) uses the rhs slice starting at
(8c+kr)*58+kc, which yields all 8 shifted rows in one contiguous view
thanks to the width padding.  The 4 groups' matmuls execute concurrently on
the PE's 32-column sub-arrays, so a chunk costs ~4 matmul streams instead
of 9 (the dense-tap formulation): ~800ns/chunk warm.  fp16 keeps 10
mantissa bits and accumulates in fp32; outputs are stored fp16 (DVE casts
on the PSUM->SBUF copy) and upcast to f32 on the host, halving output HBM
traffic (total DMA 10.0MB -> 6.8MB per core, under the ~358GB/s cap).

DMA plan: three rings (sync Q1, scalar Q10 HWDGE; gpsimd Q0 SWDGE), each
~120-145GB/s sustained, ~360GB/s HBM cap total.  Every ring issues its
transfers strictly in the order compute consumes them — front-loading all
inputs at once oversubscribes HBM and starves the piece the PE needs next
(measured: a 3.7us stall that also tripped a 3.4us half-clock HAM
throttle).  Split: sync carries img0's head rows + img3; scalar carries
weights + img1; gpsimd carries img0's tail rows + img2.  Outputs stream
out per-image as chunks are copied, in compute order (out0/out1 + img3's
odd chunks on scalar, out2 on gpsimd, img3's even chunks on sync), so the
flush after the last matmul is ~1us.  Nine dummy matmuls at the start give
the continuous PE activity that lifts the HAM clock gate (1.2 -> 2.4 GHz)
at its ~3.8us hysteresis while the first pieces are in flight; trailing
dummy matmuls (interleaved with waits on the output-DMA semaphores) keep
the clock boosted through the framework's postamble semaphore sweep, which
otherwise runs at half clock (~8us -> ~4us).
"""

from contextlib import ExitStack

import numpy as np

import concourse.bass as bass
import concourse.mybir as mybir
from concourse.bass_utils import run_bass_kernel_spmd

CFG = [[1, 2, 4, 5], [2, 3, 5, 6], [4, 5, 7, 8], [5, 6, 8, 9]]

B, C, H, W = 32, 128, 56, 56
NCORES = 8
BPC = B // NCORES            # images per core
HP, WP = H + 3, W + 2        # padded rows (1 top, 1 bottom, 1 overread), cols
XF = HP * WP                 # 3422 padded free elems per image
OF = H * W                   # 3136 output free elems per image
NPSUM = 8                    # psum banks cycled over chunks
RPC = 8                      # output rows per chunk
NCHUNK = H // RPC            # 7
NFREE = RPC * WP             # 464 matmul free dim
F32 = mybir.dt.float32
F16 = mybir.dt.float16
SLOT = [0, 1, 2, 0]          # output slot per image (3 slots)

CW = RPC * W                 # output cols per chunk (448)


def _build_nc():
    nc = bass.Bass()
    xp = nc.declare_dram_parameter("x", [C, BPC * XF], F16, isOutput=False)
    wp = nc.declare_dram_parameter("w", [C, 16 * 32], F16, isOutput=False)
    op = nc.declare_dram_parameter("out", [BPC, C, OF], F16, isOutput=True)

    with ExitStack() as ctx:
        w_tile = ctx.enter_context(nc.sbuf_tensor("w_tile", [C, 16 * 32], F16))
        xbuf = ctx.enter_context(nc.sbuf_tensor("xbuf", [C, BPC * XF], F16))
        o_slots = [ctx.enter_context(nc.sbuf_tensor(f"o_slot{i}", [C, OF], F16))
                   for i in range(3)]
        psums = [ctx.enter_context(nc.psum_tensor(f"psum{i}", [C, 512], F32))
                 for i in range(NPSUM)]

        x_sync = ctx.enter_context(nc.semaphore("x_sync"))
        x_sc = ctx.enter_context(nc.semaphore("x_sc"))
        x_gp = ctx.enter_context(nc.semaphore("x_gp"))
        o_sync = ctx.enter_context(nc.semaphore("o_sync"))
        o_sc = ctx.enter_context(nc.semaphore("o_sc"))
        o_gp = ctx.enter_context(nc.semaphore("o_gp"))
        mm_sem = ctx.enter_context(nc.semaphore("mm_sem"))
        v_sem = ctx.enter_context(nc.semaphore("v_sem"))

        block = ctx.enter_context(nc.Block(no_gpsimd_drain=True))

        def img_piece(dma, b, r_lo, r_hi, sem):
            lo, hi = b * XF + r_lo * WP, b * XF + r_hi * WP
            dma.dma_start(out=xbuf[:, lo:hi], in_=xp[:, lo:hi]).then_inc(sem, 16)

        def out_piece(dma, b, c_lo, c_hi, sem):
            dma.dma_start(
                out=op[b][:, c_lo * CW:c_hi * CW],
                in_=o_slots[SLOT[b]][:, c_lo * CW:c_hi * CW],
            ).then_inc(sem, 16)

        @block.sync
        def _(sync):
            img_piece(sync, 0, 0, 18, x_sync)       # img0 rows 0-17 (g0-1)
            img_piece(sync, 0, 18, 26, x_sync)      # img0 rows 18-25 (g2)
            img_piece(sync, 3, 0, 26, x_sync)       # img3 rows 0-25 (g21)
            img_piece(sync, 3, 26, HP, x_sync)      # img3 rows 26-58 (g24)
            for c in (0, 2, 4, 6):                  # img3 even chunks
                sync.wait_ge(v_sem, 3 * NCHUNK + c + 1)
                out_piece(sync, 3, c, c + 1, o_sync)
            sync.wait_ge(o_sync, 64)

        @block.scalar
        def _(scalar):
            scalar.dma_start(out=w_tile[:], in_=wp[:]).then_inc(x_sc, 16)
            img_piece(scalar, 1, 0, 26, x_sc)       # img1 rows 0-25 (g7)
            img_piece(scalar, 1, 26, HP, x_sc)      # img1 rows 26-58 (g10)
            scalar.wait_ge(v_sem, 4)
            out_piece(scalar, 0, 0, 4, o_sc)        # out0 c0-3
            scalar.wait_ge(v_sem, 7)
            out_piece(scalar, 0, 4, 7, o_sc)        # out0 c4-6
            scalar.wait_ge(v_sem, NCHUNK + 4)
            out_piece(scalar, 1, 0, 4, o_sc)        # out1 c0-3
            scalar.wait_ge(v_sem, 2 * NCHUNK)
            out_piece(scalar, 1, 4, 7, o_sc)        # out1 c4-6
            for c in (1, 3, 5):                     # img3 odd chunks
                scalar.wait_ge(v_sem, 3 * NCHUNK + c + 1)
                out_piece(scalar, 3, c, c + 1, o_sc)
            scalar.wait_ge(o_sc, 112)

        @block.gpsimd
        def _(gpsimd):
            img_piece(gpsimd, 0, 26, 42, x_gp)      # img0 rows 26-41 (g3)
            img_piece(gpsimd, 0, 42, HP, x_gp)      # img0 rows 42-58 (g5)
            img_piece(gpsimd, 2, 0, 26, x_gp)       # img2 rows 0-25 (g14)
            img_piece(gpsimd, 2, 26, HP, x_gp)      # img2 rows 26-58 (g17)
            gpsimd.wait_ge(v_sem, 2 * NCHUNK + 4)
            out_piece(gpsimd, 2, 0, 4, o_gp)        # out2 c0-3
            gpsimd.wait_ge(v_sem, 3 * NCHUNK)
            out_piece(gpsimd, 2, 4, 7, o_gp)        # out2 c4-6
            gpsimd.wait_ge(o_gp, 32)

        @block.tensor
        def _(tensor):
            # dummy matmuls on garbage data: continuous PE activity from the
            # earliest possible moment lifts the HAM clock gate at ~+3.8us;
            # gaps in early activity delay the boost (measured), so keep 9.
            for _ in range(9):
                tensor.matmul(
                    psums[NPSUM - 1][0:32, :NFREE],
                    w_tile[:, 0:32],
                    xbuf[:, 0:NFREE],
                    start=True, stop=True,
                    tile_position=(0, 0),
                )
            tensor.wait_ge(x_sc, 16)        # weights
            tensor.wait_ge(x_sync, 16)      # img0 rows 0-9
            # (global chunk -> input-piece semaphore threshold) gates
            gates = {2: (x_sync, 32), 3: (x_gp, 16), 5: (x_gp, 32),
                     7: (x_sc, 32), 10: (x_sc, 48),
                     14: (x_gp, 48), 17: (x_gp, 64),
                     21: (x_sync, 48), 24: (x_sync, 64)}
            for b in range(BPC):
                for c in range(NCHUNK):
                    g = NCHUNK * b + c
                    if g in gates:
                        tensor.wait_ge(*gates[g])
                    if g >= NPSUM:
                        # psum bank g%NPSUM free once chunk g-NPSUM was copied
                        tensor.wait_ge(v_sem, g - NPSUM + 1)
                    bank = psums[g % NPSUM]
                    for j in range(4):
                        for grp in range(4):
                            t = CFG[grp][j]
                            kr, kc = (t - 1) // 3, (t - 1) % 3
                            off = b * XF + (RPC * c + kr) * WP + kc
                            idx = grp * 4 + j
                            mm = tensor.matmul(
                                bank[32 * grp:32 * (grp + 1), :NFREE],
                                w_tile[:, idx * 32:(idx + 1) * 32],
                                xbuf[:, off:off + NFREE],
                                start=(j == 0),
                                stop=(j == 3),
                                tile_position=(0, 32 * grp),
                            )
                    mm.then_inc(mm_sem, 1)
            # HAM keepalive: PE activity through the output flush keeps the
            # clock boosted for the framework postamble (bank 4 is dead by
            # v_sem>=21; these results are never read).
            for sem, val in ((v_sem, 4 * NCHUNK), (o_gp, 32),
                             (o_sync, 64), (o_sc, 112)):
                tensor.wait_ge(sem, val)
                tensor.matmul(
                    psums[4][0:32, :NFREE],
                    w_tile[:, 0:32],
                    xbuf[:, 0:NFREE],
                    start=True, stop=True,
                    tile_position=(0, 0),
                )

        @block.vector
        def _(vector):
            for b in range(BPC):
                if b == 3:
                    vector.wait_ge(o_sc, 32)     # out0 done -> slot0 free
                for c in range(NCHUNK):
                    g = NCHUNK * b + c
                    vector.wait_ge(mm_sem, g + 1)
                    src = psums[g % NPSUM][:, :NFREE].rearrange(
                        "p (r w) -> p r w", w=WP)[:, :, :W]
                    dst = o_slots[SLOT[b]][:, c * CW:(c + 1) * CW].rearrange(
                        "p (r w) -> p r w", w=W)
                    vector.tensor_copy(out=dst, in_=src).then_inc(v_sem, 1)

    return nc


_NC_CACHE = None


def _get_nc():
    global _NC_CACHE
    if _NC_CACHE is None:
        _NC_CACHE = _build_nc()
    return _NC_CACHE


def _prep_weights(weights):
    """(32, 2048) grouped-sparse -> 16 [ic=128, oc=32] fp16 lhsT blocks."""
    w16 = np.zeros((C, 16 * 32), np.float32)
    for g, taps in enumerate(CFG):
        blk = np.asarray(weights[:, g * 512:(g + 1) * 512], np.float32)
        blk = blk.reshape(32, C, 4)  # [oc_in_group, ic, tap_j]
        for j in range(4):
            idx = g * 4 + j
            w16[:, idx * 32:(idx + 1) * 32] = blk[:, :, j].T
    return np.ascontiguousarray(w16.astype(np.float16))


def _prep_x(x):
    """(32,128,56,56) f32 -> per-core channel-major padded fp16 shards."""
    xpad = np.zeros((B, C, HP, WP), np.float16)
    xpad[:, :, 1:H + 1, 1:W + 1] = x.astype(np.float16)
    xs = xpad.reshape(NCORES, BPC, C, XF)
    # (core, b, c, f) -> (core, c, b*f)
    xs = np.ascontiguousarray(xs.transpose(0, 2, 1, 3)).reshape(NCORES, C, BPC * XF)
    return xs


def kernel(x, weights):
    x = np.asarray(x, np.float32)
    weights = np.asarray(weights, np.float32)

    xs = _prep_x(x)
    wflat = _prep_weights(weights)

    nc = _get_nc()
    in_maps = [{"x": xs[i], "w": wflat} for i in range(NCORES)]
    res = run_bass_kernel_spmd(nc, in_maps, core_ids=list(range(NCORES)))
    return np.concatenate(
        [res.results[i]["out"].astype(np.float32).reshape(BPC, C, H, W)
         for i in range(NCORES)],
        axis=0,
    )
